# revision 1
# baseline (speedup 1.0000x reference)
"""RNNT JointNet kernel for 8 Trainium2 NeuronCores (Bass/Tile).

Math (per reference):
    enc_proj = enc @ w_enc.T          # (B,T,H)
    dec_proj = dec @ w_dec.T          # (B,U,H)
    hidden   = gelu_tanh(enc_proj[:,:,None,:] + dec_proj[:,None,:,:] + b1)
    logits   = hidden @ w2.T          # (B,T,U,V)

Sharding: 8 cores = B(4) x U-halves(2). Each core owns (b, u_half):
full T=256, U_loc=32. Weights replicated. No collectives.

Per-core dataflow (all matmuls bf16, fp32 PSUM accumulation):
  PE:  enc_projT[h,t], dec_projT[h,u] via small matmuls (contraction d on
       partitions); then the big matmul with hiddenT tiles as the
       stationary operand: out[t(128), v(512)] += hidT[h,t_tile].T @ w2T[h,v].
  ACT: hiddenT = gelu(enc_projT + bias) where bias = dec_projT[:,u] + b1
       as a per-partition scalar -> fuses broadcast-add + bias + gelu.
  DVE: PSUM -> SBUF copies of the logits tiles.
  DMA: SBUF -> DRAM stores (natural (t,u,v) layout, 4KB contiguous rows).
"""

import numpy as np

B, T, U, D = 4, 256, 64, 512
H, V = 512, 1024
P = 128
ND = D // P  # contraction-dim chunks for projections
NH = H // P  # h chunks (contraction of the big matmul)
UL = U // 2  # U per core
N_CORES = 8

_CACHE = {}


def _build():
    import concourse.bass as bass  # noqa: F401
    import concourse.mybir as mybir
    from concourse import bacc, tile

    bf16 = mybir.dt.bfloat16
    f32 = mybir.dt.float32
    gelu = mybir.ActivationFunctionType.Gelu_apprx_tanh

    nc = bacc.Bacc(
        "TRN2",
        target_bir_lowering=False,
        debug=False,
        enable_asserts=False,
        num_devices=N_CORES,
    )

    # Inputs arrive pre-shuffled by the host into exact SBUF images
    # ([128 partitions, free]) so every load is one contiguous DMA.
    encT_d = nc.dram_tensor("encT", (P, ND * T), bf16, kind="ExternalInput")
    decT_d = nc.dram_tensor("decT", (P, ND * UL), bf16, kind="ExternalInput")
    wencT_d = nc.dram_tensor("wencT", (P, ND * H), bf16, kind="ExternalInput")
    wdecT_d = nc.dram_tensor("wdecT", (P, ND * H), bf16, kind="ExternalInput")
    w2lo_d = nc.dram_tensor("w2lo", (P, NH * 512), bf16, kind="ExternalInput")
    w2hi_d = nc.dram_tensor("w2hi", (P, NH * 512), bf16, kind="ExternalInput")
    b1c_d = nc.dram_tensor("b1c", (P, NH), f32, kind="ExternalInput")
    out_d = nc.dram_tensor("out", (T, UL, V), f32, kind="ExternalOutput")

    with tile.TileContext(nc) as tc:
        with (
            tc.tile_pool(name="const", bufs=1) as cpool,
            tc.tile_pool(name="work", bufs=1) as wpool,
            tc.tile_pool(name="hid", bufs=6) as hpool,
            tc.tile_pool(name="osb", bufs=10) as spool,
        ):
            # ---- input loads: contiguous SBUF images, one DMA each ----
            wenc_sb = cpool.tile([P, ND * H], bf16, tag="wenc")
            wdec_sb = cpool.tile([P, ND * H], bf16, tag="wdec")
            w2lo_sb = cpool.tile([P, NH * 512], bf16, tag="w2lo")
            w2hi_sb = cpool.tile([P, NH * 512], bf16, tag="w2hi")
            encT_sb = cpool.tile([P, ND * T], bf16, tag="encT")
            decT_sb = cpool.tile([P, ND * UL], bf16, tag="decT")
            b1_sb = cpool.tile([P, NH], f32, tag="b1")

            # Two HWDGE rings; only pre-gelu dependencies first (they share
            # HBM bandwidth), then w2 in per-chunk pieces in first-use order.
            nc.sync.dma_start(out=b1_sb[:], in_=b1c_d.ap()[:, :])
            nc.sync.dma_start(out=decT_sb[:], in_=decT_d.ap()[:, :])
            nc.sync.dma_start(out=wdec_sb[:], in_=wdecT_d.ap()[:, :])
            nc.scalar.dma_start(out=encT_sb[:], in_=encT_d.ap()[:, :])
            nc.scalar.dma_start(out=wenc_sb[:], in_=wencT_d.ap()[:, :])
            for i in range(NH):
                cols = slice(i * 512, (i + 1) * 512)
                nc.sync.dma_start(out=w2lo_sb[:, cols], in_=w2lo_d.ap()[:, cols])
                nc.scalar.dma_start(out=w2hi_sb[:, cols], in_=w2hi_d.ap()[:, cols])

            enc_pj = wpool.tile([P, NH * T], f32, tag="enc_pj")
            dec_pj = wpool.tile([P, NH * UL], f32, tag="dec_pj")

            # ---- projections: enc_projT[h,t], dec_projT[h,u] ----
            # (scoped PSUM pool: banks are freed for the output pool below)
            with tc.tile_pool(name="proj_ps", bufs=1, space="PSUM") as ppool:
                dec_ps = ppool.tile([P, NH * UL], f32, tag="dec_ps")  # 1 bank
                for j in range(NH):  # h slice
                    for dc in range(ND):
                        lhs_cols = slice(dc * H + j * P, dc * H + (j + 1) * P)
                        nc.tensor.matmul(
                            dec_ps[:, j * UL:(j + 1) * UL],
                            wdec_sb[:, lhs_cols],
                            decT_sb[:, dc * UL:(dc + 1) * UL],
                            start=(dc == 0), stop=(dc == ND - 1),
                        )
                for j in range(NH):
                    nc.vector.tensor_scalar_add(
                        dec_pj[:, j * UL:(j + 1) * UL],
                        dec_ps[:, j * UL:(j + 1) * UL],
                        b1_sb[:, j:j + 1],
                    )
                enc_ps = ppool.tile([P, NH * T], f32, tag="enc_ps")  # 2 banks
                for j in range(NH):
                    for dc in range(ND):
                        lhs_cols = slice(dc * H + j * P, dc * H + (j + 1) * P)
                        nc.tensor.matmul(
                            enc_ps[:, j * T:(j + 1) * T],
                            wenc_sb[:, lhs_cols],
                            encT_sb[:, dc * T:(dc + 1) * T],
                            start=(dc == 0), stop=(dc == ND - 1),
                        )
                    # per-slice copy so gelu can start before all slices finish
                    nc.vector.tensor_copy(
                        enc_pj[:, j * T:(j + 1) * T], enc_ps[:, j * T:(j + 1) * T]
                    )

            # ---- main loop over u ----
            with tc.tile_pool(name="out_ps", bufs=4, space="PSUM") as opool:
                for u in range(UL):
                    hid = hpool.tile([P, NH * T], bf16, tag="hid")
                    for i in range(NH):
                        nc.scalar.activation(
                            hid[:, i * T:(i + 1) * T],
                            enc_pj[:, i * T:(i + 1) * T],
                            gelu,
                            bias=dec_pj[:, i * UL + u: i * UL + u + 1],
                        )
                    for th in range(T // P):
                        ps = opool.tile([P, V], f32, tag="po")  # 2 PSUM banks
                        for i in range(NH):
                            lhsT = hid[:, i * T + th * P: i * T + th * P + P]
                            nc.tensor.matmul(ps[:, 0:512], lhsT,
                                             w2lo_sb[:, i * 512:(i + 1) * 512],
                                             start=(i == 0), stop=(i == NH - 1))
                            nc.tensor.matmul(ps[:, 512:V], lhsT,
                                             w2hi_sb[:, i * 512:(i + 1) * 512],
                                             start=(i == 0), stop=(i == NH - 1))
                        osb = spool.tile([P, V], f32, tag="osb")
                        nc.vector.tensor_copy(osb[:], ps[:])
                        # alternate store rings: HWDGE (sync) / SWDGE (gpsimd)
                        dma_eng = nc.sync if (u * 2 + th) % 2 == 0 else nc.gpsimd
                        dma_eng.dma_start(
                            out=out_d.ap()[th * P:(th + 1) * P, u, :], in_=osb[:]
                        )

    nc.compile()
    return nc


def _get_nc():
    if "nc" not in _CACHE:
        _CACHE["nc"] = _build()
    return _CACHE["nc"]


def _sbuf_img(mat_t):
    """[R=c*128, W] -> SBUF image [128, c*W]: img[p, c*W+w] = mat_t[c*128+p, w]."""
    r, w = mat_t.shape
    c = r // P
    return np.ascontiguousarray(
        mat_t.reshape(c, P, w).transpose(1, 0, 2).reshape(P, c * w)
    )


def _host_prep(encoder_outputs, decoder_outputs, w1, b1, w2):
    import ml_dtypes

    bf16 = ml_dtypes.bfloat16
    w_encT = _sbuf_img(w1[:, :D].T.astype(bf16))   # [D,H] -> [128, ND*H]
    w_decT = _sbuf_img(w1[:, D:].T.astype(bf16))
    w2T = w2.T.astype(bf16)                         # [H, V]
    w2lo = _sbuf_img(w2T[:, 0:512])                 # [128, NH*512]
    w2hi = _sbuf_img(w2T[:, 512:V])
    b1c = np.ascontiguousarray(b1.reshape(NH, P).T).astype(np.float32)
    in_maps = []
    for c in range(N_CORES):
        b, uh = divmod(c, 2)
        encT = _sbuf_img(encoder_outputs[b].T.astype(bf16))  # [D,T] -> [128, ND*T]
        decT = _sbuf_img(
            decoder_outputs[b, uh * UL:(uh + 1) * UL, :].T.astype(bf16)
        )
        in_maps.append({
            "encT": encT,
            "decT": decT,
            "wencT": w_encT,
            "wdecT": w_decT,
            "w2lo": w2lo,
            "w2hi": w2hi,
            "b1c": b1c,
        })
    return in_maps


def _gather(results):
    out = np.empty((B, T, U, V), dtype=np.float32)
    for c in range(N_CORES):
        b, uh = divmod(c, 2)
        out[b, :, uh * UL:(uh + 1) * UL, :] = results[c]["out"]
    return out


def kernel(encoder_outputs, decoder_outputs, w1, b1, w2):
    from concourse import bass_utils

    nc = _get_nc()
    in_maps = _host_prep(
        np.asarray(encoder_outputs), np.asarray(decoder_outputs),
        np.asarray(w1), np.asarray(b1), np.asarray(w2),
    )
    res = bass_utils.run_bass_kernel_spmd(nc, in_maps, core_ids=list(range(N_CORES)))
    return _gather(res.results)



# revision 2
# speedup vs baseline: 1.0076x; 1.0076x over previous
"""RNNT JointNet kernel for 8 Trainium2 NeuronCores (Bass/Tile).

Math (per reference):
    enc_proj = enc @ w_enc.T          # (B,T,H)
    dec_proj = dec @ w_dec.T          # (B,U,H)
    hidden   = gelu_tanh(enc_proj[:,:,None,:] + dec_proj[:,None,:,:] + b1)
    logits   = hidden @ w2.T          # (B,T,U,V)

Sharding: 8 cores = B(4) x U-halves(2). Each core owns (b, u_half):
full T=256, U_loc=32. Weights replicated. No collectives.

Per-core dataflow (all matmuls bf16, fp32 PSUM accumulation):
  PE:  warmup spin (fires the HAM clock-gate during the load phase), then
       enc_projT[h,t], dec_projT[h,u] via small matmuls; then the big
       matmul with hiddenT tiles stationary: out[t(128), v(512)] +=
       hidT[h,t_tile].T @ w2T[h,v].
  ACT: hiddenT = gelu(enc_projT + bias) where bias = dec_projT[:,u] + b1
       as a per-partition scalar -> fuses broadcast-add + bias + gelu.
  DVE: PSUM -> SBUF fp16 casts of the logits tiles.
  DMA: loads packed into 4 big transfers serialized on the sync ring
       (full HBM bandwidth, few dispatches); fp16 stores alternate
       sync/gpsimd rings; host upconverts to fp32.
"""

import numpy as np

B, T, U, D = 4, 256, 64, 512
H, V = 512, 1024
P = 128
ND = D // P  # contraction-dim chunks for projections
NH = H // P  # h chunks (contraction of the big matmul)
UL = U // 2  # U per core
N_CORES = 8
WARM_MMS = 8  # dummy N=512 matmuls to trip the HAM clock-gate early

_CACHE = {}


def _build():
    import concourse.bass as bass  # noqa: F401
    import concourse.mybir as mybir
    from concourse import bacc, tile

    bf16 = mybir.dt.bfloat16
    f16 = mybir.dt.float16
    f32 = mybir.dt.float32
    gelu = mybir.ActivationFunctionType.Gelu_apprx_tanh

    nc = bacc.Bacc(
        "TRN2",
        target_bir_lowering=False,
        debug=False,
        enable_asserts=False,
        num_devices=N_CORES,
    )

    # Inputs arrive pre-shuffled by the host into exact SBUF images
    # ([128 partitions, free]) so every load is one contiguous DMA.
    # pack_a = [decT | wdecT], pack_b = [encT | wencT].
    PA = ND * UL + ND * H      # 128 + 2048
    PB = ND * T + ND * H       # 1024 + 2048
    packa_d = nc.dram_tensor("packa", (P, PA), bf16, kind="ExternalInput")
    packb_d = nc.dram_tensor("packb", (P, PB), bf16, kind="ExternalInput")
    w2lo_d = nc.dram_tensor("w2lo", (P, NH * 512), bf16, kind="ExternalInput")
    w2hi_d = nc.dram_tensor("w2hi", (P, NH * 512), bf16, kind="ExternalInput")
    b1c_d = nc.dram_tensor("b1c", (P, NH), f32, kind="ExternalInput")
    out_d = nc.dram_tensor("out", (T, UL, V), f16, kind="ExternalOutput")

    with tile.TileContext(nc) as tc:
        with (
            tc.tile_pool(name="const", bufs=1) as cpool,
            tc.tile_pool(name="work", bufs=1) as wpool,
            tc.tile_pool(name="hid", bufs=6) as hpool,
            tc.tile_pool(name="osb", bufs=10) as spool,
        ):
            packa_sb = cpool.tile([P, PA], bf16, tag="packa")
            packb_sb = cpool.tile([P, PB], bf16, tag="packb")
            w2lo_sb = cpool.tile([P, NH * 512], bf16, tag="w2lo")
            w2hi_sb = cpool.tile([P, NH * 512], bf16, tag="w2hi")
            b1_sb = cpool.tile([P, NH], f32, tag="b1")
            warm_sb = cpool.tile([P, 512], bf16, tag="warm")

            decT_sb = packa_sb[:, 0:ND * UL]
            wdec_sb = packa_sb[:, ND * UL:PA]
            encT_sb = packb_sb[:, 0:ND * T]
            wenc_sb = packb_sb[:, ND * T:PB]

            # ---- loads: 4 big DMAs serialized on the sync ring (each gets
            # full HBM bandwidth; FIFO order = first-use order). b1 rides the
            # otherwise-idle gpsimd ring.
            nc.sync.dma_start(out=packa_sb[:], in_=packa_d.ap()[:, :])
            nc.sync.dma_start(out=packb_sb[:], in_=packb_d.ap()[:, :])
            nc.sync.dma_start(out=w2lo_sb[:], in_=w2lo_d.ap()[:, :])
            nc.sync.dma_start(out=w2hi_sb[:], in_=w2hi_d.ap()[:, :])
            nc.gpsimd.dma_start(out=b1_sb[:], in_=b1c_d.ap()[:, :])

            # ---- PE warmup: dummy matmuls on a zeroed tile keep the PE busy
            # from the end of the preamble so the HAM un-throttles to 2.4GHz
            # before the real work arrives.
            nc.vector.memset(warm_sb[:], 0)
            with tc.tile_pool(name="warm_ps", bufs=1, space="PSUM") as warmp:
                warm_ps = warmp.tile([P, 512], f32, tag="warm_ps")
                for _ in range(WARM_MMS):
                    nc.tensor.matmul(
                        warm_ps[:], warm_sb[:, 0:P], warm_sb[:],
                        start=True, stop=True,
                    )

            enc_pj = wpool.tile([P, NH * T], f32, tag="enc_pj")
            dec_pj = wpool.tile([P, NH * UL], f32, tag="dec_pj")

            # ---- projections: enc_projT[h,t], dec_projT[h,u] ----
            # (scoped PSUM pool: banks are freed for the output pool below)
            with tc.tile_pool(name="proj_ps", bufs=1, space="PSUM") as ppool:
                dec_ps = ppool.tile([P, NH * UL], f32, tag="dec_ps")  # 1 bank
                for j in range(NH):  # h slice
                    for dc in range(ND):
                        lhs_cols = slice(dc * H + j * P, dc * H + (j + 1) * P)
                        nc.tensor.matmul(
                            dec_ps[:, j * UL:(j + 1) * UL],
                            wdec_sb[:, lhs_cols],
                            decT_sb[:, dc * UL:(dc + 1) * UL],
                            start=(dc == 0), stop=(dc == ND - 1),
                        )
                for j in range(NH):
                    nc.vector.tensor_scalar_add(
                        dec_pj[:, j * UL:(j + 1) * UL],
                        dec_ps[:, j * UL:(j + 1) * UL],
                        b1_sb[:, j:j + 1],
                    )
                enc_ps = ppool.tile([P, NH * T], f32, tag="enc_ps")  # 2 banks
                for j in range(NH):
                    for dc in range(ND):
                        lhs_cols = slice(dc * H + j * P, dc * H + (j + 1) * P)
                        nc.tensor.matmul(
                            enc_ps[:, j * T:(j + 1) * T],
                            wenc_sb[:, lhs_cols],
                            encT_sb[:, dc * T:(dc + 1) * T],
                            start=(dc == 0), stop=(dc == ND - 1),
                        )
                    # per-slice copy so gelu can start before all slices finish
                    nc.vector.tensor_copy(
                        enc_pj[:, j * T:(j + 1) * T], enc_ps[:, j * T:(j + 1) * T]
                    )

            # ---- main loop over u ----
            with tc.tile_pool(name="out_ps", bufs=4, space="PSUM") as opool:
                for u in range(UL):
                    hid = hpool.tile([P, NH * T], bf16, tag="hid")
                    for i in range(NH):
                        nc.scalar.activation(
                            hid[:, i * T:(i + 1) * T],
                            enc_pj[:, i * T:(i + 1) * T],
                            gelu,
                            bias=dec_pj[:, i * UL + u: i * UL + u + 1],
                        )
                    for th in range(T // P):
                        ps = opool.tile([P, V], f32, tag="po")  # 2 PSUM banks
                        for i in range(NH):
                            lhsT = hid[:, i * T + th * P: i * T + th * P + P]
                            nc.tensor.matmul(ps[:, 0:512], lhsT,
                                             w2lo_sb[:, i * 512:(i + 1) * 512],
                                             start=(i == 0), stop=(i == NH - 1))
                            nc.tensor.matmul(ps[:, 512:V], lhsT,
                                             w2hi_sb[:, i * 512:(i + 1) * 512],
                                             start=(i == 0), stop=(i == NH - 1))
                        osb = spool.tile([P, V], f16, tag="osb")
                        nc.vector.tensor_copy(osb[:], ps[:])
                        # alternate store rings: HWDGE (sync) / SWDGE (gpsimd)
                        dma_eng = nc.sync if (u * 2 + th) % 2 == 0 else nc.gpsimd
                        dma_eng.dma_start(
                            out=out_d.ap()[th * P:(th + 1) * P, u, :], in_=osb[:]
                        )

    nc.compile()
    return nc


def _get_nc():
    if "nc" not in _CACHE:
        _CACHE["nc"] = _build()
    return _CACHE["nc"]


def _sbuf_img(mat_t):
    """[R=c*128, W] -> SBUF image [128, c*W]: img[p, c*W+w] = mat_t[c*128+p, w]."""
    r, w = mat_t.shape
    c = r // P
    return np.ascontiguousarray(
        mat_t.reshape(c, P, w).transpose(1, 0, 2).reshape(P, c * w)
    )


def _host_prep(encoder_outputs, decoder_outputs, w1, b1, w2):
    import ml_dtypes

    bf16 = ml_dtypes.bfloat16
    w_encT = _sbuf_img(w1[:, :D].T.astype(bf16))   # [D,H] -> [128, ND*H]
    w_decT = _sbuf_img(w1[:, D:].T.astype(bf16))
    w2T = w2.T.astype(bf16)                         # [H, V]
    w2lo = _sbuf_img(w2T[:, 0:512])                 # [128, NH*512]
    w2hi = _sbuf_img(w2T[:, 512:V])
    b1c = np.ascontiguousarray(b1.reshape(NH, P).T).astype(np.float32)
    in_maps = []
    for c in range(N_CORES):
        b, uh = divmod(c, 2)
        encT = _sbuf_img(encoder_outputs[b].T.astype(bf16))  # [D,T] -> [128, ND*T]
        decT = _sbuf_img(
            decoder_outputs[b, uh * UL:(uh + 1) * UL, :].T.astype(bf16)
        )
        packa = np.concatenate([decT, w_decT], axis=1)
        packb = np.concatenate([encT, w_encT], axis=1)
        in_maps.append({
            "packa": np.ascontiguousarray(packa),
            "packb": np.ascontiguousarray(packb),
            "w2lo": w2lo,
            "w2hi": w2hi,
            "b1c": b1c,
        })
    return in_maps


def _gather(results):
    out = np.empty((B, T, U, V), dtype=np.float32)
    for c in range(N_CORES):
        b, uh = divmod(c, 2)
        out[b, :, uh * UL:(uh + 1) * UL, :] = np.asarray(
            results[c]["out"], dtype=np.float32
        )
    return out


def kernel(encoder_outputs, decoder_outputs, w1, b1, w2):
    from concourse import bass_utils

    nc = _get_nc()
    in_maps = _host_prep(
        np.asarray(encoder_outputs), np.asarray(decoder_outputs),
        np.asarray(w1), np.asarray(b1), np.asarray(w2),
    )
    res = bass_utils.run_bass_kernel_spmd(nc, in_maps, core_ids=list(range(N_CORES)))
    return _gather(res.results)
